# revision 36
# baseline (speedup 1.0000x reference)
"""Fused linear + cross-entropy loss on 8 Trainium2 NeuronCores.

loss = sum_t [logsumexp(h_t @ W^T) - (h_t @ W^T)[target_t]] / n_valid

Sharding: data-parallel over tokens (8192 tokens -> 1024/core). Each core
streams the full weight matrix once (as wT [D, V]) and keeps its hidden
slice (hT [D, 1024]) resident in SBUF. Per [128-token x 500-vocab] logits
tile (PSUM, fp32 accumulated over D via 16 matmuls):
  - ScalarE: exp with fused accum_out -> per-token partial sum-of-exp
  - VectorE: scalar_tensor_tensor (iota == target) * logits with fused
    accum_out -> per-token target-logit partial (one-hot gather)
Per-token loss = ln(sum_j sumexp_j) - sum_j tgt_j computed on device;
the host applies the IGNORE_INDEX mask and takes the mean.

No cross-core collectives are needed: every core owns its tokens fully.
"""

import numpy as np
from contextlib import ExitStack

import concourse.bacc as bacc
import concourse.bass as bass
import concourse.mybir as mybir
import concourse.tile as tile
from concourse.bass_utils import run_bass_kernel_spmd
from concourse.tile import add_dep_helper

IGNORE_INDEX = -100

# Full problem dims (hardcoded; harness contract).
VOCAB = 32000
D = 2048
B = 4
S = 2048
N_CORES = 8
N_TOK = B * S                  # 8192
T_CORE = N_TOK // N_CORES      # 1024 tokens per core
P = 128                        # SBUF partitions

# fp32r (= tfloat32, 10-bit mantissa): fp32-width data streamed through the
# PE at bf16 rate (1 cycle/row for moving free dim >= 256), vs 4 cycles/row
# for true fp32. The BIR verifier requires fp32r matmul operands to be
# produced already-rounded, so the host pre-rounds (RNE to 13 dropped bits)
# and hT/wT are declared float32r end to end.
MM_DTYPE = mybir.dt.float32r


def _round_tf32(x):
    """Round fp32 array to tfloat32 (10 mantissa bits), round-nearest-even."""
    u = np.ascontiguousarray(x, dtype=np.float32).view(np.uint32)
    lsb = (u >> np.uint32(13)) & np.uint32(1)
    u = (u + np.uint32(0x0FFF) + lsb) & np.uint32(0xFFFFE000)
    return u.view(np.float32)


def build_ce_program(t_core=T_CORE, vocab=VOCAB, d=D, vchunk=500,
                     mm_dtype=MM_DTYPE, w_bufs=2, psum_bufs=7,
                     compile_program=False):
    """Build the single-core Bass/Tile program (SPMD across cores)."""
    assert t_core % P == 0 and d % P == 0 and vocab % vchunk == 0
    m_tiles = t_core // P
    kd = d // P
    nj = vocab // vchunk
    f32 = mybir.dt.float32

    # Bacc (not raw Bass): its compile() pipeline splits multi-sync-wait
    # instructions into event-semaphore chains, which the TRN2 ISA requires
    # (at most one wait per engine instruction).
    nc = bacc.Bacc("TRN2", target_bir_lowering=False, debug=False)
    hT = nc.declare_dram_parameter("hT", [d, t_core], mm_dtype, isOutput=False)
    wT = nc.declare_dram_parameter("wT", [d, vocab], mm_dtype, isOutput=False)
    # Single const input: cols [0:vchunk] = iota row, rest = target shifts
    # (one DMA -> one semaphore wait on the consumer; the ISA caps per-
    # instruction sync waits).
    consts_in = nc.declare_dram_parameter(
        "consts", [P, vchunk + m_tiles * nj], f32, isOutput=False)
    loss_out = nc.declare_dram_parameter("loss_tok", [P, m_tiles], f32,
                                         isOutput=True)

    with tile.TileContext(nc) as tc, ExitStack() as ctx:
        const_pool = ctx.enter_context(tc.tile_pool(name="const", bufs=1))
        h_pool = ctx.enter_context(tc.tile_pool(name="h", bufs=1))
        w_pool = ctx.enter_context(tc.tile_pool(name="w", bufs=w_bufs))
        psum_pool = ctx.enter_context(
            tc.tile_pool(name="psum", bufs=psum_bufs, space="PSUM"))
        stat_pool = ctx.enter_context(tc.tile_pool(name="stat", bufs=1))

        consts_t = const_pool.tile([P, vchunk + m_tiles * nj], f32,
                                   tag="consts")
        nc.sync.dma_start(consts_t[:], consts_in[:])
        iota_t = consts_t[:, 0:vchunk]
        tgt_t = consts_t[:, vchunk:vchunk + m_tiles * nj]
        # Pre-touch consts on DVE: the TensorScalarPtr ISA slot carries only
        # ONE sync wait, so the const-DMA wait must land here, leaving the
        # first STT in the main loop with just its psum dependency.
        touch_t = const_pool.tile([P, 1], f32, tag="touch")
        nc.vector.tensor_copy(touch_t[:], consts_t[:, 0:1])

        # Resident hidden slice: kd tiles of [128 (D sub), t_core tokens].
        hT_r = hT.rearrange("(k p) t -> k p t", p=P)
        hts = []
        for k in range(kd):
            ht = h_pool.tile([P, t_core], mm_dtype, tag=f"ht{k}")
            nc.sync.dma_start(ht[:], hT_r[k])
            hts.append(ht)

        # Per-(token, vocab-chunk) partial stats; col = m * nj + j.
        sum_stats = stat_pool.tile([P, m_tiles * nj], f32, tag="sumexp")
        tgt_stats = stat_pool.tile([P, m_tiles * nj], f32, tag="tgtacc")
        # Throwaway main-output target for the STT gather ops: each op writes
        # its own column via a step-0 (broadcast) AP, so no WAW dep ever
        # forms and every ACT/DVE instruction keeps to the single ISA
        # sync-wait slot.
        stt_dump = stat_pool.tile([P, m_tiles * nj], f32, tag="sttdump")
        # Per-tile ACT-side absorber target: a cheap ScalarE copy reads the
        # STT's column first, so the engine clock observes the DVE tick and
        # the in-place exp needs only its own WAW wait (Tile does not elide
        # transitive cross-engine deps).
        act_absorb = stat_pool.tile([P, m_tiles * nj], f32, tag="absorb")

        # PE-side scratch bank for "touch" matmuls (distinct column per
        # chunk -> no WAW): each absorbs the weight-chunk DMA wait on the PE
        # so real matmuls keep to the single LDWEIGHTS sync-wait slot.
        touch_pool = ctx.enter_context(
            tc.tile_pool(name="ptouch", bufs=1, space="PSUM"))
        n_sub = 4 if kd % 4 == 0 else 1
        touch_w = nj * n_sub
        assert touch_w * 2 * 4 <= 2048, "touch columns must fit one PSUM bank"
        ps_touch = touch_pool.tile([1, touch_w * 2], f32, tag="ptouch")

        # The weight stream is split into 4 sub-DMAs per vocab chunk, each
        # with its own tag and bufs=2: the slot-reuse distance is then
        # exactly 8 DMA issues, so Tile's round-robin (mod 8) puts a slot's
        # previous writer on the SAME DMAHW lane as the new DMA -- the WAW
        # wait and the lane-bookkeeping wait collapse into one, keeping
        # every DMA within the 2-sync-wait pseudo-DMA ISA limit.
        ksub = kd // n_sub
        wT_r = wT.rearrange("(k p) v -> p k v", p=P)  # [128, kd, vocab]
        last_mm = {}
        for j in range(nj):
            # The pseudo-DMA ISA form carries a single sync wait, which the
            # DMAHW lane-reuse wait occupies. Absorb the PE slot-release
            # wait (weights of chunk j-w_bufs fully consumed) on an SP nop
            # ordered before this chunk's weight DMAs.
            if j - w_bufs in last_mm:
                nop = nc.sync.nop()
                add_dep_helper(nop.ins, last_mm[j - w_bufs].ins, sync=True,
                               reason="absorb wt slot-release PE wait")
            else:
                nop = None
            subs = []
            for s in range(n_sub):
                wt = w_pool.tile([P, ksub, vchunk], mm_dtype, tag=f"wt{s}")
                dma = nc.sync.dma_start(
                    wt[:], wT_r[:, s * ksub:(s + 1) * ksub,
                                j * vchunk:(j + 1) * vchunk])
                if nop is not None:
                    add_dep_helper(dma.ins, nop.ins, sync=False,
                                   reason="order after absorber nop")
                # Touch matmul: absorbs this sub-DMA's wait on the PE so
                # real matmuls keep to the single LDWEIGHTS sync-wait slot.
                # (fp32r matmuls need an even moving free dim, so N=2.)
                tcol = 2 * (j * n_sub + s)
                nc.tensor.matmul(ps_touch[0:1, tcol:tcol + 2],
                                 lhsT=wt[:, 0, 0:1], rhs=wt[:, 0, 0:2],
                                 start=True, stop=True)
                subs.append(wt)
            for m in range(m_tiles):
                ps = psum_pool.tile([P, vchunk], f32, tag="ps")
                for k in range(kd):
                    mm = nc.tensor.matmul(
                        ps[:],
                        lhsT=hts[k][:, m * P:(m + 1) * P],
                        rhs=subs[k // ksub][:, k % ksub, :],
                        start=(k == 0),
                        stop=(k == kd - 1),
                    )
                last_mm[j] = mm
                col = m * nj + j
                # Target-logit gather first (reads raw logits), then exp
                # IN-PLACE on the psum tile (WAR dep serializes it after the
                # gather; its accum_out is the per-token partial sum-of-exp).
                nc.vector.scalar_tensor_tensor(
                    stt_dump[:, col:col + 1].broadcast_to([P, vchunk]),
                    in0=iota_t,
                    scalar=tgt_t[:, col:col + 1],
                    in1=ps[:],
                    op0=mybir.AluOpType.is_equal,
                    op1=mybir.AluOpType.mult,
                    accum_out=tgt_stats[:, col:col + 1])
                nc.scalar.copy(act_absorb[:, col:col + 1],
                               stt_dump[:, col:col + 1])
                nc.scalar.activation(
                    ps[:], ps[:], mybir.ActivationFunctionType.Exp,
                    accum_out=sum_stats[:, col:col + 1])

        # Epilogue: per-token lse = ln(sum over chunks), tgt = sum over
        # chunks, loss = lse - tgt.
        sum_tot = stat_pool.tile([P, m_tiles], f32, tag="sumtot")
        tgt_tot = stat_pool.tile([P, m_tiles], f32, tag="tgttot")
        lse_t = stat_pool.tile([P, m_tiles], f32, tag="lse")
        loss_t = stat_pool.tile([P, m_tiles], f32, tag="loss")
        for m in range(m_tiles):
            nc.vector.reduce_sum(sum_tot[:, m:m + 1],
                                 sum_stats[:, m * nj:(m + 1) * nj],
                                 axis=mybir.AxisListType.X)
            nc.vector.reduce_sum(tgt_tot[:, m:m + 1],
                                 tgt_stats[:, m * nj:(m + 1) * nj],
                                 axis=mybir.AxisListType.X)
        nc.scalar.activation(lse_t[:], sum_tot[:],
                             mybir.ActivationFunctionType.Ln)
        # DVE pre-touch of lse so the final subtract needs only one wait.
        touch2_t = const_pool.tile([P, 1], f32, tag="touch2")
        nc.vector.tensor_copy(touch2_t[:], lse_t[:, 0:1])
        sub_i = nc.vector.tensor_sub(loss_t[:], lse_t[:], tgt_tot[:])
        # Absorb the DVE wait on an SP nop so the output DMA keeps to the
        # single pseudo-DMA sync-wait slot (lane wait).
        nop_out = nc.sync.nop()
        add_dep_helper(nop_out.ins, sub_i.ins, sync=True,
                       reason="absorb loss DVE wait")
        out_dma = nc.sync.dma_start(loss_out[:], loss_t[:])
        add_dep_helper(out_dma.ins, nop_out.ins, sync=False,
                       reason="order after absorber nop")

    if compile_program:
        nc.compile()
    return nc


_PROGRAM_CACHE = {}


def _get_program():
    key = "full"
    if key not in _PROGRAM_CACHE:
        _PROGRAM_CACHE[key] = build_ce_program(compile_program=True)
    return _PROGRAM_CACHE[key]


def make_in_maps(weight, hidden, targets, t_core=T_CORE, vocab=VOCAB, d=D,
                 vchunk=500, n_cores=N_CORES):
    """Host-side sharding: transpose operands, shard tokens, build per-core
    target-shift tables."""
    m_tiles = t_core // P
    nj = vocab // vchunk
    weight = np.ascontiguousarray(np.asarray(weight, dtype=np.float32))
    hidden = np.asarray(hidden, dtype=np.float32).reshape(-1, d)
    flat_t = np.asarray(targets).reshape(-1)
    safe_t = np.clip(flat_t, 0, vocab - 1).astype(np.float32)

    wT = _round_tf32(np.ascontiguousarray(weight.T))   # [d, vocab]
    hT = _round_tf32(np.ascontiguousarray(hidden.T))   # [d, n_tok]
    iota = np.broadcast_to(np.arange(vchunk, dtype=np.float32), (P, vchunk))
    offs = (np.arange(nj, dtype=np.float32) * vchunk)

    in_maps = []
    for c in range(n_cores):
        tgt_c = safe_t[c * t_core:(c + 1) * t_core].reshape(m_tiles, P)
        # shift[p, m, j] = target(token m*128+p) - j*vchunk
        shift = tgt_c.T[:, :, None] - offs[None, None, :]
        consts = np.concatenate(
            [iota, shift.reshape(P, m_tiles * nj)], axis=1).astype(np.float32)
        in_maps.append({
            "hT": np.ascontiguousarray(hT[:, c * t_core:(c + 1) * t_core]),
            "wT": wT,
            "consts": np.ascontiguousarray(consts),
        })
    return in_maps


def combine_outputs(results, flat_t, t_core=T_CORE, n_cores=N_CORES):
    """Gather per-token losses, apply IGNORE_INDEX mask, mean-reduce."""
    m_tiles = t_core // P
    # loss_tok[c][p, m] is the loss of global token c*t_core + m*128 + p.
    per_tok = np.concatenate([
        np.asarray(r["loss_tok"]).T.reshape(-1) for r in results])
    valid = flat_t != IGNORE_INDEX
    total = np.where(valid, per_tok, 0.0).sum(dtype=np.float32)
    n_valid = np.float32(valid.sum())
    loss = total / n_valid if n_valid > 0 else total
    return np.float32(loss)


def kernel(weight, hidden, targets):
    in_maps = make_in_maps(weight, hidden, targets)
    nc = _get_program()
    res = run_bass_kernel_spmd(nc, in_maps, list(range(N_CORES)))
    flat_t = np.asarray(targets).reshape(-1)
    return combine_outputs(res.results, flat_t)


# revision 39
# speedup vs baseline: 1.0673x; 1.0673x over previous
"""Fused linear + cross-entropy loss on 8 Trainium2 NeuronCores.

loss = sum_t [logsumexp(h_t @ W^T) - (h_t @ W^T)[target_t]] / n_valid

Sharding: data-parallel over tokens (8192 tokens -> 1024/core). Each core
streams the full weight matrix once (as wT [D, V]) and keeps its hidden
slice (hT [D, 1024]) resident in SBUF. Per [128-token x 500-vocab] logits
tile (PSUM, fp32 accumulated over D via 16 matmuls):
  - ScalarE: exp with fused accum_out -> per-token partial sum-of-exp
  - VectorE: scalar_tensor_tensor (iota == target) * logits with fused
    accum_out -> per-token target-logit partial (one-hot gather)
Per-token loss = ln(sum_j sumexp_j) - sum_j tgt_j computed on device;
the host applies the IGNORE_INDEX mask and takes the mean.

No cross-core collectives are needed: every core owns its tokens fully.
"""

import numpy as np
from contextlib import ExitStack

import concourse.bacc as bacc
import concourse.bass as bass
import concourse.mybir as mybir
import concourse.tile as tile
from concourse.bass_utils import run_bass_kernel_spmd
from concourse.tile import add_dep_helper

IGNORE_INDEX = -100

# Full problem dims (hardcoded; harness contract).
VOCAB = 32000
D = 2048
B = 4
S = 2048
N_CORES = 8
N_TOK = B * S                  # 8192
T_CORE = N_TOK // N_CORES      # 1024 tokens per core
P = 128                        # SBUF partitions

# fp32r (= tfloat32, 10-bit mantissa): fp32-width data streamed through the
# PE at bf16 rate (1 cycle/row for moving free dim >= 256), vs 4 cycles/row
# for true fp32. The BIR verifier requires fp32r matmul operands to be
# produced already-rounded, so the host pre-rounds (RNE to 13 dropped bits)
# and hT/wT are declared float32r end to end.
MM_DTYPE = mybir.dt.float32r


def _round_tf32(x):
    """Round fp32 array to tfloat32 (10 mantissa bits), round-nearest-even."""
    u = np.ascontiguousarray(x, dtype=np.float32).view(np.uint32)
    lsb = (u >> np.uint32(13)) & np.uint32(1)
    u = (u + np.uint32(0x0FFF) + lsb) & np.uint32(0xFFFFE000)
    return u.view(np.float32)


def _to_mm_host_dtype(x, mm_dtype):
    """Convert a host fp32 array to the matmul operand's host dtype."""
    if mm_dtype == mybir.dt.float32r:
        return _round_tf32(x)
    return np.asarray(x).astype(mybir.dt.np(mm_dtype))


def build_ce_program(t_core=T_CORE, vocab=VOCAB, d=D, vchunk=500,
                     mm_dtype=MM_DTYPE, w_bufs=2, psum_bufs=7,
                     compile_program=False):
    """Build the single-core Bass/Tile program (SPMD across cores)."""
    assert t_core % P == 0 and d % P == 0 and vocab % vchunk == 0
    m_tiles = t_core // P
    kd = d // P
    nj = vocab // vchunk
    f32 = mybir.dt.float32

    # Bacc (not raw Bass): its compile() pipeline splits multi-sync-wait
    # instructions into event-semaphore chains, which the TRN2 ISA requires
    # (at most one wait per engine instruction).
    nc = bacc.Bacc("TRN2", target_bir_lowering=False, debug=False)
    hT = nc.declare_dram_parameter("hT", [d, t_core], mm_dtype, isOutput=False)
    wT = nc.declare_dram_parameter("wT", [d, vocab], mm_dtype, isOutput=False)
    # Single const input: cols [0:vchunk] = iota row, rest = target shifts
    # (one DMA -> one semaphore wait on the consumer; the ISA caps per-
    # instruction sync waits).
    consts_in = nc.declare_dram_parameter(
        "consts", [P, vchunk + m_tiles * nj], f32, isOutput=False)
    loss_out = nc.declare_dram_parameter("loss_tok", [P, m_tiles], f32,
                                         isOutput=True)

    with tile.TileContext(nc) as tc, ExitStack() as ctx:
        const_pool = ctx.enter_context(tc.tile_pool(name="const", bufs=1))
        h_pool = ctx.enter_context(tc.tile_pool(name="h", bufs=1))
        w_pool = ctx.enter_context(tc.tile_pool(name="w", bufs=w_bufs))
        psum_pool = ctx.enter_context(
            tc.tile_pool(name="psum", bufs=psum_bufs, space="PSUM"))
        stat_pool = ctx.enter_context(tc.tile_pool(name="stat", bufs=1))

        consts_t = const_pool.tile([P, vchunk + m_tiles * nj], f32,
                                   tag="consts")
        nc.sync.dma_start(consts_t[:], consts_in[:])
        iota_t = consts_t[:, 0:vchunk]
        tgt_t = consts_t[:, vchunk:vchunk + m_tiles * nj]
        # Pre-touch consts on DVE: the TensorScalarPtr ISA slot carries only
        # ONE sync wait, so the const-DMA wait must land here, leaving the
        # first STT in the main loop with just its psum dependency.
        touch_t = const_pool.tile([P, 1], f32, tag="touch")
        nc.vector.tensor_copy(touch_t[:], consts_t[:, 0:1])

        # Resident hidden slice: kd tiles of [128 (D sub), t_core tokens].
        hT_r = hT.rearrange("(k p) t -> k p t", p=P)
        hts = []
        for k in range(kd):
            ht = h_pool.tile([P, t_core], mm_dtype, tag=f"ht{k}")
            nc.sync.dma_start(ht[:], hT_r[k])
            hts.append(ht)

        # Per-(token, vocab-chunk) partial stats; col = m * nj + j.
        sum_stats = stat_pool.tile([P, m_tiles * nj], f32, tag="sumexp")
        tgt_stats = stat_pool.tile([P, m_tiles * nj], f32, tag="tgtacc")
        # Throwaway main-output target for the STT gather ops: each op writes
        # its own column via a step-0 (broadcast) AP, so no WAW dep ever
        # forms and every ACT/DVE instruction keeps to the single ISA
        # sync-wait slot.
        stt_dump = stat_pool.tile([P, m_tiles * nj], f32, tag="sttdump")
        # Per-tile ACT-side absorber target: a cheap ScalarE copy reads the
        # STT's column first, so the engine clock observes the DVE tick and
        # the in-place exp needs only its own WAW wait (Tile does not elide
        # transitive cross-engine deps).
        act_absorb = stat_pool.tile([P, m_tiles * nj], f32, tag="absorb")

        # PE-side scratch bank for "touch" matmuls (distinct column per
        # chunk -> no WAW): each absorbs the weight-chunk DMA wait on the PE
        # so real matmuls keep to the single LDWEIGHTS sync-wait slot.
        touch_pool = ctx.enter_context(
            tc.tile_pool(name="ptouch", bufs=1, space="PSUM"))
        n_sub = 4 if kd % 4 == 0 else 1
        touch_w = nj * n_sub
        assert touch_w * 2 * 4 <= 2048, "touch columns must fit one PSUM bank"
        ps_touch = touch_pool.tile([1, touch_w * 2], f32, tag="ptouch")

        # The weight stream is split into 4 sub-DMAs per vocab chunk, each
        # with its own tag and bufs=2: the slot-reuse distance is then
        # exactly 8 DMA issues, so Tile's round-robin (mod 8) puts a slot's
        # previous writer on the SAME DMAHW lane as the new DMA -- the WAW
        # wait and the lane-bookkeeping wait collapse into one, keeping
        # every DMA within the 2-sync-wait pseudo-DMA ISA limit.
        ksub = kd // n_sub
        wT_r = wT.rearrange("(k p) v -> p k v", p=P)  # [128, kd, vocab]
        last_mm = {}
        for j in range(nj):
            # The pseudo-DMA ISA form carries a single sync wait, which the
            # DMAHW lane-reuse wait occupies. Absorb the PE slot-release
            # wait (weights of chunk j-w_bufs fully consumed) on an SP nop
            # ordered before this chunk's weight DMAs.
            if j - w_bufs in last_mm:
                nop = nc.sync.nop()
                add_dep_helper(nop.ins, last_mm[j - w_bufs].ins, sync=True,
                               reason="absorb wt slot-release PE wait")
            else:
                nop = None
            subs = []
            for s in range(n_sub):
                wt = w_pool.tile([P, ksub, vchunk], mm_dtype, tag=f"wt{s}")
                dma = nc.sync.dma_start(
                    wt[:], wT_r[:, s * ksub:(s + 1) * ksub,
                                j * vchunk:(j + 1) * vchunk])
                if nop is not None:
                    add_dep_helper(dma.ins, nop.ins, sync=False,
                                   reason="order after absorber nop")
                # Touch matmul: absorbs this sub-DMA's wait on the PE so
                # real matmuls keep to the single LDWEIGHTS sync-wait slot.
                # (fp32r matmuls need an even moving free dim, so N=2.)
                tcol = 2 * (j * n_sub + s)
                nc.tensor.matmul(ps_touch[0:1, tcol:tcol + 2],
                                 lhsT=wt[:, 0, 0:1], rhs=wt[:, 0, 0:2],
                                 start=True, stop=True)
                subs.append(wt)
            for m in range(m_tiles):
                ps = psum_pool.tile([P, vchunk], f32, tag="ps")
                for k in range(kd):
                    mm = nc.tensor.matmul(
                        ps[:],
                        lhsT=hts[k][:, m * P:(m + 1) * P],
                        rhs=subs[k // ksub][:, k % ksub, :],
                        start=(k == 0),
                        stop=(k == kd - 1),
                    )
                last_mm[j] = mm
                col = m * nj + j
                # Target-logit gather first (reads raw logits), then exp
                # IN-PLACE on the psum tile (WAR dep serializes it after the
                # gather; its accum_out is the per-token partial sum-of-exp).
                nc.vector.scalar_tensor_tensor(
                    stt_dump[:, col:col + 1].broadcast_to([P, vchunk]),
                    in0=iota_t,
                    scalar=tgt_t[:, col:col + 1],
                    in1=ps[:],
                    op0=mybir.AluOpType.is_equal,
                    op1=mybir.AluOpType.mult,
                    accum_out=tgt_stats[:, col:col + 1])
                nc.scalar.copy(act_absorb[:, col:col + 1],
                               stt_dump[:, col:col + 1])
                nc.scalar.activation(
                    ps[:], ps[:], mybir.ActivationFunctionType.Exp,
                    accum_out=sum_stats[:, col:col + 1])

        # Epilogue: per-token lse = ln(sum over chunks), tgt = sum over
        # chunks, loss = lse - tgt.
        sum_tot = stat_pool.tile([P, m_tiles], f32, tag="sumtot")
        tgt_tot = stat_pool.tile([P, m_tiles], f32, tag="tgttot")
        lse_t = stat_pool.tile([P, m_tiles], f32, tag="lse")
        loss_t = stat_pool.tile([P, m_tiles], f32, tag="loss")
        for m in range(m_tiles):
            nc.vector.reduce_sum(sum_tot[:, m:m + 1],
                                 sum_stats[:, m * nj:(m + 1) * nj],
                                 axis=mybir.AxisListType.X)
            nc.vector.reduce_sum(tgt_tot[:, m:m + 1],
                                 tgt_stats[:, m * nj:(m + 1) * nj],
                                 axis=mybir.AxisListType.X)
        nc.scalar.activation(lse_t[:], sum_tot[:],
                             mybir.ActivationFunctionType.Ln)
        # DVE pre-touch of lse so the final subtract needs only one wait.
        touch2_t = const_pool.tile([P, 1], f32, tag="touch2")
        nc.vector.tensor_copy(touch2_t[:], lse_t[:, 0:1])
        sub_i = nc.vector.tensor_sub(loss_t[:], lse_t[:], tgt_tot[:])
        # Absorb the DVE wait on an SP nop so the output DMA keeps to the
        # single pseudo-DMA sync-wait slot (lane wait).
        nop_out = nc.sync.nop()
        add_dep_helper(nop_out.ins, sub_i.ins, sync=True,
                       reason="absorb loss DVE wait")
        out_dma = nc.sync.dma_start(loss_out[:], loss_t[:])
        add_dep_helper(out_dma.ins, nop_out.ins, sync=False,
                       reason="order after absorber nop")

    if compile_program:
        nc.compile()
    return nc


_PROGRAM_CACHE = {}


def _get_program():
    key = "full"
    if key not in _PROGRAM_CACHE:
        _PROGRAM_CACHE[key] = build_ce_program(compile_program=True)
    return _PROGRAM_CACHE[key]


def make_in_maps(weight, hidden, targets, t_core=T_CORE, vocab=VOCAB, d=D,
                 vchunk=500, n_cores=N_CORES, mm_dtype=MM_DTYPE):
    """Host-side sharding: transpose operands, shard tokens, build per-core
    target-shift tables."""
    m_tiles = t_core // P
    nj = vocab // vchunk
    weight = np.ascontiguousarray(np.asarray(weight, dtype=np.float32))
    hidden = np.asarray(hidden, dtype=np.float32).reshape(-1, d)
    flat_t = np.asarray(targets).reshape(-1)
    safe_t = np.clip(flat_t, 0, vocab - 1).astype(np.float32)

    wT = _to_mm_host_dtype(np.ascontiguousarray(weight.T), mm_dtype)  # [d,v]
    hT = _to_mm_host_dtype(np.ascontiguousarray(hidden.T), mm_dtype)  # [d,t]
    iota = np.broadcast_to(np.arange(vchunk, dtype=np.float32), (P, vchunk))
    offs = (np.arange(nj, dtype=np.float32) * vchunk)

    in_maps = []
    for c in range(n_cores):
        tgt_c = safe_t[c * t_core:(c + 1) * t_core].reshape(m_tiles, P)
        # shift[p, m, j] = target(token m*128+p) - j*vchunk
        shift = tgt_c.T[:, :, None] - offs[None, None, :]
        consts = np.concatenate(
            [iota, shift.reshape(P, m_tiles * nj)], axis=1).astype(np.float32)
        in_maps.append({
            "hT": np.ascontiguousarray(hT[:, c * t_core:(c + 1) * t_core]),
            "wT": wT,
            "consts": np.ascontiguousarray(consts),
        })
    return in_maps


def combine_outputs(results, flat_t, t_core=T_CORE, n_cores=N_CORES):
    """Gather per-token losses, apply IGNORE_INDEX mask, mean-reduce."""
    m_tiles = t_core // P
    # loss_tok[c][p, m] is the loss of global token c*t_core + m*128 + p.
    per_tok = np.concatenate([
        np.asarray(r["loss_tok"]).T.reshape(-1) for r in results])
    valid = flat_t != IGNORE_INDEX
    total = np.where(valid, per_tok, 0.0).sum(dtype=np.float32)
    n_valid = np.float32(valid.sum())
    loss = total / n_valid if n_valid > 0 else total
    return np.float32(loss)


def kernel(weight, hidden, targets):
    in_maps = make_in_maps(weight, hidden, targets)
    nc = _get_program()
    res = run_bass_kernel_spmd(nc, in_maps, list(range(N_CORES)))
    flat_t = np.asarray(targets).reshape(-1)
    return combine_outputs(res.results, flat_t)


# revision 40
# speedup vs baseline: 1.0687x; 1.0013x over previous
"""Fused linear + cross-entropy loss on 8 Trainium2 NeuronCores.

loss = sum_t [logsumexp(h_t @ W^T) - (h_t @ W^T)[target_t]] / n_valid

Sharding: data-parallel over tokens (8192 tokens -> 1024/core). Each core
streams the full weight matrix once (as wT [D, V]) and keeps its hidden
slice (hT [D, 1024]) resident in SBUF. Per [128-token x 500-vocab] logits
tile (PSUM, fp32 accumulated over D via 16 matmuls):
  - ScalarE: exp with fused accum_out -> per-token partial sum-of-exp
  - VectorE: scalar_tensor_tensor (iota == target) * logits with fused
    accum_out -> per-token target-logit partial (one-hot gather)
Per-token loss = ln(sum_j sumexp_j) - sum_j tgt_j computed on device;
the host applies the IGNORE_INDEX mask and takes the mean.

No cross-core collectives are needed: every core owns its tokens fully.
"""

import numpy as np
from contextlib import ExitStack

import concourse.bacc as bacc
import concourse.bass as bass
import concourse.mybir as mybir
import concourse.tile as tile
from concourse.bass_utils import run_bass_kernel_spmd
from concourse.tile import add_dep_helper

IGNORE_INDEX = -100

# Full problem dims (hardcoded; harness contract).
VOCAB = 32000
D = 2048
B = 4
S = 2048
N_CORES = 8
N_TOK = B * S                  # 8192
T_CORE = N_TOK // N_CORES      # 1024 tokens per core
P = 128                        # SBUF partitions

# Matmul operand dtype. Measured on HW (8 cores, full size):
#   bfloat16: 1.789 ms, loss rel err 1.2e-06  <- default
#   float32r: 1.909 ms, loss rel err 8.7e-08  (tfloat32; use if tighter
#             accuracy is ever needed; host pre-rounds RNE as the BIR
#             verifier requires fp32r operands produced already-rounded)
# Errors in the scalar loss stay tiny because per-token quantization noise
# averages down over 8192 tokens. PE streams 1 column/cycle for both.
MM_DTYPE = mybir.dt.bfloat16


def _round_tf32(x):
    """Round fp32 array to tfloat32 (10 mantissa bits), round-nearest-even."""
    u = np.ascontiguousarray(x, dtype=np.float32).view(np.uint32)
    lsb = (u >> np.uint32(13)) & np.uint32(1)
    u = (u + np.uint32(0x0FFF) + lsb) & np.uint32(0xFFFFE000)
    return u.view(np.float32)


def _to_mm_host_dtype(x, mm_dtype):
    """Convert a host fp32 array to the matmul operand's host dtype."""
    if mm_dtype == mybir.dt.float32r:
        return _round_tf32(x)
    return np.asarray(x).astype(mybir.dt.np(mm_dtype))


def build_ce_program(t_core=T_CORE, vocab=VOCAB, d=D, vchunk=500,
                     mm_dtype=MM_DTYPE, w_bufs=2, psum_bufs=7,
                     compile_program=False):
    """Build the single-core Bass/Tile program (SPMD across cores)."""
    assert t_core % P == 0 and d % P == 0 and vocab % vchunk == 0
    m_tiles = t_core // P
    kd = d // P
    nj = vocab // vchunk
    f32 = mybir.dt.float32

    # Bacc (not raw Bass): its compile() pipeline splits multi-sync-wait
    # instructions into event-semaphore chains, which the TRN2 ISA requires
    # (at most one wait per engine instruction).
    nc = bacc.Bacc("TRN2", target_bir_lowering=False, debug=False)
    hT = nc.declare_dram_parameter("hT", [d, t_core], mm_dtype, isOutput=False)
    wT = nc.declare_dram_parameter("wT", [d, vocab], mm_dtype, isOutput=False)
    # Single const input: cols [0:vchunk] = iota row, rest = target shifts
    # (one DMA -> one semaphore wait on the consumer; the ISA caps per-
    # instruction sync waits).
    consts_in = nc.declare_dram_parameter(
        "consts", [P, vchunk + m_tiles * nj], f32, isOutput=False)
    loss_out = nc.declare_dram_parameter("loss_tok", [P, m_tiles], f32,
                                         isOutput=True)

    with tile.TileContext(nc) as tc, ExitStack() as ctx:
        const_pool = ctx.enter_context(tc.tile_pool(name="const", bufs=1))
        h_pool = ctx.enter_context(tc.tile_pool(name="h", bufs=1))
        w_pool = ctx.enter_context(tc.tile_pool(name="w", bufs=w_bufs))
        psum_pool = ctx.enter_context(
            tc.tile_pool(name="psum", bufs=psum_bufs, space="PSUM"))
        stat_pool = ctx.enter_context(tc.tile_pool(name="stat", bufs=1))

        consts_t = const_pool.tile([P, vchunk + m_tiles * nj], f32,
                                   tag="consts")
        nc.sync.dma_start(consts_t[:], consts_in[:])
        iota_t = consts_t[:, 0:vchunk]
        tgt_t = consts_t[:, vchunk:vchunk + m_tiles * nj]
        # Pre-touch consts on DVE: the TensorScalarPtr ISA slot carries only
        # ONE sync wait, so the const-DMA wait must land here, leaving the
        # first STT in the main loop with just its psum dependency.
        touch_t = const_pool.tile([P, 1], f32, tag="touch")
        nc.vector.tensor_copy(touch_t[:], consts_t[:, 0:1])

        # Resident hidden slice: kd tiles of [128 (D sub), t_core tokens].
        hT_r = hT.rearrange("(k p) t -> k p t", p=P)
        hts = []
        for k in range(kd):
            ht = h_pool.tile([P, t_core], mm_dtype, tag=f"ht{k}")
            nc.sync.dma_start(ht[:], hT_r[k])
            hts.append(ht)

        # Per-(token, vocab-chunk) partial stats; col = m * nj + j.
        sum_stats = stat_pool.tile([P, m_tiles * nj], f32, tag="sumexp")
        tgt_stats = stat_pool.tile([P, m_tiles * nj], f32, tag="tgtacc")
        # Throwaway main-output target for the STT gather ops: each op writes
        # its own column via a step-0 (broadcast) AP, so no WAW dep ever
        # forms and every ACT/DVE instruction keeps to the single ISA
        # sync-wait slot.
        stt_dump = stat_pool.tile([P, m_tiles * nj], f32, tag="sttdump")
        # Per-tile ACT-side absorber target: a cheap ScalarE copy reads the
        # STT's column first, so the engine clock observes the DVE tick and
        # the in-place exp needs only its own WAW wait (Tile does not elide
        # transitive cross-engine deps).
        act_absorb = stat_pool.tile([P, m_tiles * nj], f32, tag="absorb")

        # PE-side scratch bank for "touch" matmuls (distinct column per
        # chunk -> no WAW): each absorbs the weight-chunk DMA wait on the PE
        # so real matmuls keep to the single LDWEIGHTS sync-wait slot.
        touch_pool = ctx.enter_context(
            tc.tile_pool(name="ptouch", bufs=1, space="PSUM"))
        n_sub = 4 if kd % 4 == 0 else 1
        touch_w = nj * n_sub
        assert touch_w * 2 * 4 <= 2048, "touch columns must fit one PSUM bank"
        ps_touch = touch_pool.tile([1, touch_w * 2], f32, tag="ptouch")

        # The weight stream is split into 4 sub-DMAs per vocab chunk, each
        # with its own tag and bufs=2: the slot-reuse distance is then
        # exactly 8 DMA issues, so Tile's round-robin (mod 8) puts a slot's
        # previous writer on the SAME DMAHW lane as the new DMA -- the WAW
        # wait and the lane-bookkeeping wait collapse into one, keeping
        # every DMA within the 2-sync-wait pseudo-DMA ISA limit.
        ksub = kd // n_sub
        wT_r = wT.rearrange("(k p) v -> p k v", p=P)  # [128, kd, vocab]
        last_mm = {}
        for j in range(nj):
            # The pseudo-DMA ISA form carries a single sync wait, which the
            # DMAHW lane-reuse wait occupies. Absorb the PE slot-release
            # wait (weights of chunk j-w_bufs fully consumed) on an SP nop
            # ordered before this chunk's weight DMAs.
            if j - w_bufs in last_mm:
                nop = nc.sync.nop()
                add_dep_helper(nop.ins, last_mm[j - w_bufs].ins, sync=True,
                               reason="absorb wt slot-release PE wait")
            else:
                nop = None
            subs = []
            for s in range(n_sub):
                wt = w_pool.tile([P, ksub, vchunk], mm_dtype, tag=f"wt{s}")
                dma = nc.sync.dma_start(
                    wt[:], wT_r[:, s * ksub:(s + 1) * ksub,
                                j * vchunk:(j + 1) * vchunk])
                if nop is not None:
                    add_dep_helper(dma.ins, nop.ins, sync=False,
                                   reason="order after absorber nop")
                # Touch matmul: absorbs this sub-DMA's wait on the PE so
                # real matmuls keep to the single LDWEIGHTS sync-wait slot.
                # (fp32r matmuls need an even moving free dim, so N=2.)
                tcol = 2 * (j * n_sub + s)
                nc.tensor.matmul(ps_touch[0:1, tcol:tcol + 2],
                                 lhsT=wt[:, 0, 0:1], rhs=wt[:, 0, 0:2],
                                 start=True, stop=True)
                subs.append(wt)
            for m in range(m_tiles):
                ps = psum_pool.tile([P, vchunk], f32, tag="ps")
                for k in range(kd):
                    mm = nc.tensor.matmul(
                        ps[:],
                        lhsT=hts[k][:, m * P:(m + 1) * P],
                        rhs=subs[k // ksub][:, k % ksub, :],
                        start=(k == 0),
                        stop=(k == kd - 1),
                    )
                last_mm[j] = mm
                col = m * nj + j
                # Target-logit gather first (reads raw logits), then exp
                # IN-PLACE on the psum tile (WAR dep serializes it after the
                # gather; its accum_out is the per-token partial sum-of-exp).
                nc.vector.scalar_tensor_tensor(
                    stt_dump[:, col:col + 1].broadcast_to([P, vchunk]),
                    in0=iota_t,
                    scalar=tgt_t[:, col:col + 1],
                    in1=ps[:],
                    op0=mybir.AluOpType.is_equal,
                    op1=mybir.AluOpType.mult,
                    accum_out=tgt_stats[:, col:col + 1])
                nc.scalar.copy(act_absorb[:, col:col + 1],
                               stt_dump[:, col:col + 1])
                nc.scalar.activation(
                    ps[:], ps[:], mybir.ActivationFunctionType.Exp,
                    accum_out=sum_stats[:, col:col + 1])

        # Epilogue: per-token lse = ln(sum over chunks), tgt = sum over
        # chunks, loss = lse - tgt.
        sum_tot = stat_pool.tile([P, m_tiles], f32, tag="sumtot")
        tgt_tot = stat_pool.tile([P, m_tiles], f32, tag="tgttot")
        lse_t = stat_pool.tile([P, m_tiles], f32, tag="lse")
        loss_t = stat_pool.tile([P, m_tiles], f32, tag="loss")
        for m in range(m_tiles):
            nc.vector.reduce_sum(sum_tot[:, m:m + 1],
                                 sum_stats[:, m * nj:(m + 1) * nj],
                                 axis=mybir.AxisListType.X)
            nc.vector.reduce_sum(tgt_tot[:, m:m + 1],
                                 tgt_stats[:, m * nj:(m + 1) * nj],
                                 axis=mybir.AxisListType.X)
        nc.scalar.activation(lse_t[:], sum_tot[:],
                             mybir.ActivationFunctionType.Ln)
        # DVE pre-touch of lse so the final subtract needs only one wait.
        touch2_t = const_pool.tile([P, 1], f32, tag="touch2")
        nc.vector.tensor_copy(touch2_t[:], lse_t[:, 0:1])
        sub_i = nc.vector.tensor_sub(loss_t[:], lse_t[:], tgt_tot[:])
        # Absorb the DVE wait on an SP nop so the output DMA keeps to the
        # single pseudo-DMA sync-wait slot (lane wait).
        nop_out = nc.sync.nop()
        add_dep_helper(nop_out.ins, sub_i.ins, sync=True,
                       reason="absorb loss DVE wait")
        out_dma = nc.sync.dma_start(loss_out[:], loss_t[:])
        add_dep_helper(out_dma.ins, nop_out.ins, sync=False,
                       reason="order after absorber nop")

    if compile_program:
        nc.compile()
    return nc


_PROGRAM_CACHE = {}


def _get_program():
    key = "full"
    if key not in _PROGRAM_CACHE:
        _PROGRAM_CACHE[key] = build_ce_program(compile_program=True)
    return _PROGRAM_CACHE[key]


def make_in_maps(weight, hidden, targets, t_core=T_CORE, vocab=VOCAB, d=D,
                 vchunk=500, n_cores=N_CORES, mm_dtype=MM_DTYPE):
    """Host-side sharding: transpose operands, shard tokens, build per-core
    target-shift tables."""
    m_tiles = t_core // P
    nj = vocab // vchunk
    weight = np.ascontiguousarray(np.asarray(weight, dtype=np.float32))
    hidden = np.asarray(hidden, dtype=np.float32).reshape(-1, d)
    flat_t = np.asarray(targets).reshape(-1)
    safe_t = np.clip(flat_t, 0, vocab - 1).astype(np.float32)

    wT = _to_mm_host_dtype(np.ascontiguousarray(weight.T), mm_dtype)  # [d,v]
    hT = _to_mm_host_dtype(np.ascontiguousarray(hidden.T), mm_dtype)  # [d,t]
    iota = np.broadcast_to(np.arange(vchunk, dtype=np.float32), (P, vchunk))
    offs = (np.arange(nj, dtype=np.float32) * vchunk)

    in_maps = []
    for c in range(n_cores):
        tgt_c = safe_t[c * t_core:(c + 1) * t_core].reshape(m_tiles, P)
        # shift[p, m, j] = target(token m*128+p) - j*vchunk
        shift = tgt_c.T[:, :, None] - offs[None, None, :]
        consts = np.concatenate(
            [iota, shift.reshape(P, m_tiles * nj)], axis=1).astype(np.float32)
        in_maps.append({
            "hT": np.ascontiguousarray(hT[:, c * t_core:(c + 1) * t_core]),
            "wT": wT,
            "consts": np.ascontiguousarray(consts),
        })
    return in_maps


def combine_outputs(results, flat_t, t_core=T_CORE, n_cores=N_CORES):
    """Gather per-token losses, apply IGNORE_INDEX mask, mean-reduce."""
    m_tiles = t_core // P
    # loss_tok[c][p, m] is the loss of global token c*t_core + m*128 + p.
    per_tok = np.concatenate([
        np.asarray(r["loss_tok"]).T.reshape(-1) for r in results])
    valid = flat_t != IGNORE_INDEX
    total = np.where(valid, per_tok, 0.0).sum(dtype=np.float32)
    n_valid = np.float32(valid.sum())
    loss = total / n_valid if n_valid > 0 else total
    return np.float32(loss)


def kernel(weight, hidden, targets):
    in_maps = make_in_maps(weight, hidden, targets)
    nc = _get_program()
    res = run_bass_kernel_spmd(nc, in_maps, list(range(N_CORES)))
    flat_t = np.asarray(targets).reshape(-1)
    return combine_outputs(res.results, flat_t)
